# revision 20
# baseline (speedup 1.0000x reference)
"""Trainium2 Bass kernel for nn_AvgPoolingModel (embedding avg-pool + tiny MLP).

Model:  emb = table[batch]           # [B, L, 300] gather
        pooled = emb.sum(1) / lens   # [B, 300]
        h1 = relu(pooled @ W1.T + b1)
        h2 = relu(h1 @ W2.T + b2)
        y  = (h2 @ W3.T + b3)[:, 0]  # [B]

Sharding: data-parallel over B across 8 cores (512 rows/core); folded
embedding table + MLP weights replicated per core.

Key transform: W1 is folded into the table HOST-side
(table2 = emb_table @ W1.T, [100000, 150] f32), which is exact
(pooled @ W1.T == (1/len)*sum table2[idx]) and cuts gathered bytes from
1200 to 600 per index.  Rows are padded to 192 f32 (768 B) for the 256 B
stride/payload rule of `dma_gather`.

Gather strategy (`dma_gather`, int16 indices, batched descriptors):
  - vocab split into 4 sub-ranges of 25000 rows so local indices fit
    int16; each sub-table gets 32 zero rows (pad reads spread across
    them to avoid single-HBM-channel congestion).
  - gathered list position j lands in partition j%128, slot j//128; the
    host orders each call's list so batch row p's indices stay in
    partition p.
  - HW limit (probed): <=1024 indices per call; each (row-tile,
    sub-range) splits into G balanced occurrence groups of width W<=8.
  - calls round-robin over 4 SWDGE queues (a single queue serializes
    descriptor generation behind DMA completion: 12.7us vs 1.9us/call).
  - batch rows are clustered into row-tiles by sub-range count profile
    (greedy), shrinking the rectangle width (max per-partition count)
    and with it the zero-row padding; the row permutation is undone
    host-side on the [B] output.

Per core: 4 row-tiles of gathers land packed into [128, 64*192] f32 ring
buffers (2 bufs); one DVE strided reduce per buffer (cols [:150])
collapses it into the row-tile accumulator.  Epilogue per tile: scale by
1/len, + b1 (rank-1 ones matmul in PSUM with an identity-matmul copy of
the scaled acc), relu, then the tiny W2/W3 matmuls with on-chip
transposes.
"""

import numpy as np

import concourse.bass as bass
import concourse.mybir as mybir
from concourse.tile import TileContext

VOCAB, EMB = 100000, 300
B, L = 4096, 200
H1, H2 = 150, 150
NCORES = 8
BC = B // NCORES  # rows per core
P = 128
NT = BC // P      # row-tiles per core
NSUB = 4
SUBSZ = VOCAB // NSUB
NZROW = 256           # spread pad reads over this many zero rows
SUBROWS = SUBSZ + NZROW
ZROW = SUBSZ
ESZ = 192             # table row stride (f32 elements), 768 B
PAYL = 192            # gathered payload per index (f32): full 768 B rows
                      # (HBM random access is transaction-bound: smaller
                      # payloads measured SLOWER, 608 B: 474us vs 768 B: 406us)
MAXW = 8              # max slots per partition per dma_gather call
SLOTBUF = 64          # ring buffer capacity in slots
NQ = 4                # SWDGE queues

F32 = mybir.dt.float32
I32 = mybir.dt.int32
I16 = mybir.dt.int16


def _dma_gather_raw(gp, out_ap, in_ap, idxs_ap, num_idxs, elem_size,
                    elem_step, queue_num=0):
    """dma_gather with payload not a multiple of 256 B (bass asserts that,
    but it is a transpose-only restriction: the non-transpose Q7
    descriptor generator sizes packets in raw bytes; only the stride --
    the idx multiplier -- is 256 B-granular in hardware).  Mirrors
    bass.dma_gather's non-transpose path."""
    from concourse import ap_utils
    from concourse.bass import MemorySpace, exact_div, round_up_to_multiple

    gp._assert_queue_num(queue_num)
    assert idxs_ap.dtype == mybir.dt.int16
    assert in_ap.space == MemorySpace.DRAM
    assert in_ap.dtype == out_ap.dtype
    assert idxs_ap.space == MemorySpace.SBUF
    assert out_ap.space == MemorySpace.SBUF
    assert ap_utils.ap_is_contiguous(out_ap.ap[1:])
    assert ap_utils.ap_is_contiguous(idxs_ap.ap[1:])
    assert num_idxs % 128 == 0
    assert out_ap.ap[0][1] * out_ap.ap[1][1] == round_up_to_multiple(num_idxs, 128)
    assert out_ap.ap[-1][1] == elem_size
    assert in_ap.ap[0][0] == elem_step
    dsz = mybir.dt.size(in_ap.dtype)
    stride_bytes_256 = exact_div(elem_step * dsz, 256)
    assert 0 < stride_bytes_256 < 256
    _in_ap = gp.lower_ap_dma(in_ap, for_custom_bir_dma=True)
    _idxs_ap = gp.lower_ap(idxs_ap)
    _out_ap = gp.lower_ap(out_ap)
    return gp.add_instruction(
        mybir.InstDMAGatherAnt(
            name=gp.bass.get_next_instruction_name(),
            ins=[*_in_ap, _idxs_ap, gp.lower_val_access(gp.to_reg(num_idxs))],
            outs=[_out_ap],
            transpose=False,
            num_idxs=num_idxs,
            elem_size=elem_size,
            stride_bytes_256=stride_bytes_256,
            gen_mode=0,
            single_packet=True,
            queue_num=queue_num,
            sbuf_tokens_per_rank=0,
            sbuf_free_dim_per_rank=0,
            sbuf_free_dim_pad_per_rank=0,
            sbuf_byte_offset=0,
        )
    )


def _cluster_rows(cnt):
    """Greedy: assign rows (count vectors [BC, NSUB]) to NT tiles of P rows,
    minimizing sum over tiles of per-sub-range max count.  Processing
    peaky rows first makes tile specializations line up across cores,
    which matters because call widths take a max over cores.  Returns
    perm (row order, tile-major)."""
    order = np.argsort(-cnt.max(axis=1), kind="stable")
    tiles = [[] for _ in range(NT)]
    tmax = np.zeros((NT, NSUB), np.int32)
    for r in order:
        best, bcost = None, None
        for t in range(NT):
            if len(tiles[t]) >= P:
                continue
            cost = int(np.maximum(tmax[t], cnt[r]).sum() - tmax[t].sum())
            if bcost is None or cost < bcost:
                best, bcost = t, cost
        tiles[best].append(r)
        tmax[best] = np.maximum(tmax[best], cnt[r])

    return np.concatenate([np.asarray(t, np.int64) for t in tiles])


def compute_plan(batch):
    """Host-side gather plan from the full [B, L] int index matrix.

    Returns (calls, tot, gidx_cores, perms):
      calls: tuple of (t, s, W, idx_off) in issue order; 128*W idx/call.
      tot:   per-partition int16 elements of the concatenated index tile.
      gidx_cores: list of NCORES [128, tot] int16 index tiles.
      perms: list of NCORES [BC] row permutations (device row i holds
             original row perm[i] of that core's slice).
    """
    batch = np.asarray(batch)
    sub = (batch // SUBSZ).astype(np.int8)
    loc = (batch - (batch // SUBSZ) * SUBSZ).astype(np.int16)

    perms = []
    groups = []   # groups[c][t][s][p] = array of local indices
    maxc = np.zeros((NT, NSUB), np.int32)
    for c in range(NCORES):
        r0 = c * BC
        cnt = np.stack(
            [(sub[r0:r0 + BC] == s).sum(axis=1) for s in range(NSUB)], axis=1
        ).astype(np.int32)
        perm = _cluster_rows(cnt)
        perms.append(perm)
        gc = [[[None] * P for _ in range(NSUB)] for _ in range(NT)]
        for t in range(NT):
            for p in range(P):
                rr = r0 + perm[t * P + p]
                rs, rl = sub[rr], loc[rr]
                for s in range(NSUB):
                    g = np.sort(rl[rs == s])
                    gc[t][s][p] = g
                    if len(g) > maxc[t, s]:
                        maxc[t, s] = len(g)
        groups.append(gc)

    calls = []
    off = 0
    for t in range(NT):
        for s in range(NSUB):
            mc = int(maxc[t, s])
            G = -(-mc // MAXW)  # ceil
            for g in range(G):
                W = min(MAXW, mc - MAXW * g)  # exact: sum of widths == maxc
                calls.append((t, s, g, G, W, off))
                off += 8 * W  # (128*W)/16 int16 elems per partition
    tot = off

    gidx_cores = []
    for c in range(NCORES):
        gidx = np.empty((P, tot), np.int16)
        padctr = 0
        for (t, s, g, G, W, ioff) in calls:
            arr = np.empty((P, W), np.int16)
            for p in range(P):
                gp = groups[c][t][s][p][MAXW * g: MAXW * g + W]
                n = len(gp)
                arr[p, :n] = gp
                npad = W - n
                if npad:
                    arr[p, n:] = ZROW + (padctr + np.arange(npad)) % NZROW
                    padctr += npad
            flat = arr.T.reshape(-1)               # flat[j], j = m*128 + p
            tile16 = flat.reshape(8 * W, 16).T     # [16, 8W]
            gidx[:, ioff: ioff + 8 * W] = np.tile(tile16, (8, 1))
        gidx_cores.append(gidx)

    call_keys = tuple((t, s, W, ioff) for (t, s, g, G, W, ioff) in calls)
    return call_keys, tot, gidx_cores, perms


def build_nc(calls, tot, repeat=None, gather_only=False):
    """Build the per-core Bass kernel for a gather plan.

    repeat=None: the real kernel.  repeat=R: the gather+reduce+MLP body is
    wrapped in a hardware For_i loop executing R times (identical work per
    iteration) — used only for wall-clock timing, where slope over R
    isolates HW exec time from the ~100ms axon dispatch overhead.
    """
    from concourse import bacc
    from concourse import library_config
    from concourse.masks import make_identity

    nc = bacc.Bacc("TRN2", target_bir_lowering=False, debug=False,
                   num_swdge_queues=NQ)

    gidx_d = nc.dram_tensor("gidx", [P, tot], I16, kind="ExternalInput")
    recip_d = nc.dram_tensor("recip", [BC], F32, kind="ExternalInput")
    tab_d = nc.dram_tensor("tab2", [NSUB * SUBROWS, ESZ], F32, kind="ExternalInput")
    b1_d = nc.dram_tensor("b1", [H1], F32, kind="ExternalInput")
    w2t_d = nc.dram_tensor("w2t", [H1, H2], F32, kind="ExternalInput")
    b2_d = nc.dram_tensor("b2", [H2], F32, kind="ExternalInput")
    w3t_d = nc.dram_tensor("w3t", [H2, 1], F32, kind="ExternalInput")
    b3_d = nc.dram_tensor("b3", [1], F32, kind="ExternalInput")
    y_d = nc.dram_tensor("y", [BC], F32, kind="ExternalOutput")

    # pack calls into ring buffers (per row-tile, <= SLOTBUF slots each)
    bufplans = {t: [] for t in range(NT)}  # t -> list of list[(s, W, ioff)]
    for (t, s, w, ioff) in calls:
        plans = bufplans[t]
        if not plans or sum(x[1] for x in plans[-1]) + w > SLOTBUF:
            plans.append([])
        plans[-1].append((s, w, ioff))

    with TileContext(nc) as tc:
        with (
            tc.tile_pool(name="const", bufs=1) as cpool,
            tc.tile_pool(name="gat", bufs=1) as gpool,
            tc.tile_pool(name="ring", bufs=3) as rpool,
            tc.tile_pool(name="work", bufs=2) as wpool,
            tc.tile_pool(name="psum", bufs=1, space="PSUM") as ppool,
            tc.tile_pool(name="psum2", bufs=2, space="PSUM") as ppool2,
        ):
            # ---- index tile (loaded first: it gates the gather stream) --
            gidx_sb = gpool.tile([P, tot], I16, tag="gidx", name="gidx_sb")
            tile_off = [tot]
            for (t, s, w, ioff) in calls:
                if t == 1:
                    tile_off[0] = ioff
                    break
            nc.sync.dma_start(out=gidx_sb[:, : tile_off[0]],
                              in_=gidx_d[:, : tile_off[0]])
            nc.sync.dma_start(out=gidx_sb[:, tile_off[0]:],
                              in_=gidx_d[:, tile_off[0]:])

            accs = []
            for t in range(NT):
                accs.append(gpool.tile([P, H1], F32, tag=f"acc{t}", name=f"acc{t}"))

            # ---- one-time constants -------------------------------------
            identity = cpool.tile([P, P], F32)
            make_identity(nc, identity[:])
            ones_row = cpool.tile([1, P], F32)
            nc.vector.memset(ones_row[:], 1.0)
            # gpsimd work above uses the standard library; everything after
            # this point on gpsimd is dma_gather (mlp library).
            nc.gpsimd.load_library(library_config.mlp)

            w2t_sb = cpool.tile([75, 2 * H2], F32)    # 2 K-chunks of W2.T
            for c in range(2):
                nc.sync.dma_start(
                    out=w2t_sb[:, c * H2:(c + 1) * H2],
                    in_=w2t_d[c * 75:(c + 1) * 75, :],
                )
            w3t_sb = cpool.tile([75, 2], F32)         # 2 K-chunks of W3.T
            for c in range(2):
                nc.sync.dma_start(
                    out=w3t_sb[:, c:c + 1], in_=w3t_d[c * 75:(c + 1) * 75, :]
                )
            b1_sb = cpool.tile([1, H1], F32)
            nc.sync.dma_start(out=b1_sb[:], in_=b1_d[None, :])
            b2_sb = cpool.tile([1, H2], F32)
            nc.sync.dma_start(out=b2_sb[:], in_=b2_d[None, :])
            b3_sb = cpool.tile([1, 1], F32)
            nc.sync.dma_start(out=b3_sb[:], in_=b3_d[None, :])

            recip_sb = cpool.tile([P, NT], F32)
            nc.sync.dma_start(
                out=recip_sb[:], in_=recip_d.ap().rearrange("(t p) -> p t", p=P)
            )
            out_sb = cpool.tile([P, NT], F32)

            qctr = [0]

            # ---- per-row-tile: batched gathers + packed DVE reduces -----
            def gather_tile(t, it=""):
                first = True
                for bi, plan in enumerate(bufplans[t]):
                    nslots = sum(w for (_s, w, _o) in plan)
                    ring = rpool.tile([P, SLOTBUF * PAYL], F32, tag="ring",
                                      name=f"ring{t}_{bi}{it}")
                    col = 0
                    for (s, w, ioff) in plan:
                        _dma_gather_raw(
                            nc.gpsimd,
                            ring[:, col * PAYL:(col + w) * PAYL].rearrange(
                                "p (w e) -> p w e", e=PAYL
                            ),
                            tab_d[s * SUBROWS:(s + 1) * SUBROWS, :],
                            gidx_sb[:, ioff: ioff + 8 * w],
                            P * w,
                            PAYL,
                            ESZ,
                            queue_num=qctr[0] % NQ,
                        )
                        qctr[0] += 1
                        col += w
                    # one contiguous fold halves the slot count before the
                    # strided reduce (slots' cols [H1:PAYL] are all zero)
                    n = nslots
                    if n > 2:
                        h = n // 2
                        nc.vector.tensor_add(
                            out=ring[:, : h * PAYL],
                            in0=ring[:, : h * PAYL],
                            in1=ring[:, (n - h) * PAYL: n * PAYL],
                        )
                        n = n - h
                    rview = ring[:, : n * PAYL].rearrange(
                        "p (w e) -> p e w", e=PAYL
                    )[:, :H1, :]
                    if first:
                        nc.vector.reduce_sum(
                            accs[t][:], rview, axis=mybir.AxisListType.X
                        )
                        first = False
                    else:
                        part = wpool.tile([P, H1], F32, tag="part",
                                          name=f"part{t}_{bi}{it}")
                        nc.vector.reduce_sum(
                            part[:], rview, axis=mybir.AxisListType.X
                        )
                        nc.vector.tensor_add(
                            out=accs[t][:], in0=accs[t][:], in1=part[:]
                        )

            # ---- per-row-tile epilogue: scale, +b1, relu, W2, W3 --------
            def epilogue_tile(t, it=""):
                acc = accs[t]
                scaled = wpool.tile([P, H1], F32, tag="scaled",
                                    name=f"scaled{t}{it}")
                nc.vector.tensor_scalar_mul(
                    scaled[:], acc[:], recip_sb[:, t:t + 1]
                )

                # h1 = relu(scaled + b1): PSUM gets identity-matmul copy of
                # scaled, then the rank-1 ones x b1 matmul adds the bias.
                h1_ps = ppool.tile([P, H1], F32, tag="h1", name=f"h1ps{t}{it}")
                nc.tensor.matmul(
                    out=h1_ps[:], lhsT=identity[:], rhs=scaled[:],
                    start=True, stop=False,
                )
                nc.tensor.matmul(
                    out=h1_ps[:], lhsT=ones_row[:], rhs=b1_sb[:],
                    start=False, stop=True,
                )
                h1_sb = wpool.tile([P, H1], F32, tag="h1sb", name=f"h1sb{t}{it}")
                nc.scalar.activation(
                    h1_sb[:], h1_ps[:], mybir.ActivationFunctionType.Relu
                )

                h1t = wpool.tile([75, 2 * P], F32, tag="h1t", name=f"h1t{t}{it}")
                for c in range(2):
                    t1_ps = ppool2.tile([75, P], F32, tag="tps",
                                        name=f"t1{t}_{c}{it}")
                    nc.tensor.transpose(
                        out=t1_ps[:], in_=h1_sb[:, c * 75:(c + 1) * 75],
                        identity=identity[:],
                    )
                    nc.scalar.copy(h1t[:, c * P:(c + 1) * P], t1_ps[:])

                h2_ps = ppool.tile([P, H2], F32, tag="h2", name=f"h2ps{t}{it}")
                for c in range(2):
                    nc.tensor.matmul(
                        out=h2_ps[:],
                        lhsT=h1t[:, c * P:(c + 1) * P],
                        rhs=w2t_sb[:, c * H2:(c + 1) * H2],
                        start=(c == 0), stop=False,
                    )
                nc.tensor.matmul(
                    out=h2_ps[:], lhsT=ones_row[:], rhs=b2_sb[:],
                    start=False, stop=True,
                )
                h2_sb = wpool.tile([P, H2], F32, tag="h2sb", name=f"h2sb{t}{it}")
                nc.scalar.activation(
                    h2_sb[:], h2_ps[:], mybir.ActivationFunctionType.Relu
                )

                h2t = wpool.tile([75, 2 * P], F32, tag="h2t", name=f"h2t{t}{it}")
                for c in range(2):
                    t2_ps = ppool2.tile([75, P], F32, tag="tps",
                                        name=f"t2{t}_{c}{it}")
                    nc.tensor.transpose(
                        out=t2_ps[:], in_=h2_sb[:, c * 75:(c + 1) * 75],
                        identity=identity[:],
                    )
                    nc.scalar.copy(h2t[:, c * P:(c + 1) * P], t2_ps[:])

                y_ps = ppool.tile([P, 1], F32, tag="y", name=f"yps{t}{it}")
                for c in range(2):
                    nc.tensor.matmul(
                        out=y_ps[:],
                        lhsT=h2t[:, c * P:(c + 1) * P],
                        rhs=w3t_sb[:, c:c + 1],
                        start=(c == 0), stop=False,
                    )
                nc.tensor.matmul(
                    out=y_ps[:], lhsT=ones_row[:], rhs=b3_sb[:],
                    start=False, stop=True,
                )
                nc.scalar.copy(out_sb[:, t:t + 1], y_ps[:])

            def body(it=""):
                if gather_only:
                    # gathers + a token DVE consumer per buffer (copy of one
                    # column) so Tile keeps real DMA-completion deps without
                    # the full reduce cost.
                    for t in range(NT):
                        for bi, plan in enumerate(bufplans[t]):
                            ring = rpool.tile([P, SLOTBUF * PAYL], F32,
                                              tag="ring", name=f"ring{t}_{bi}{it}")
                            col = 0
                            for (s, w, ioff) in plan:
                                _dma_gather_raw(
                                    nc.gpsimd,
                                    ring[:, col * PAYL:(col + w) * PAYL].rearrange(
                                        "p (w e) -> p w e", e=PAYL
                                    ),
                                    tab_d[s * SUBROWS:(s + 1) * SUBROWS, :],
                                    gidx_sb[:, ioff: ioff + 8 * w],
                                    P * w, PAYL, ESZ,
                                    queue_num=qctr[0] % NQ,
                                )
                                qctr[0] += 1
                                col += w
                            nc.vector.tensor_copy(
                                out=out_sb[:, t:t + 1],
                                in_=ring[:, (col - 1) * PAYL:(col - 1) * PAYL + 1],
                            )
                    nc.sync.dma_start(
                        out=y_d.ap().rearrange("(t p) -> p t", p=P), in_=out_sb[:]
                    )
                    return
                for t in range(NT):
                    gather_tile(t, it)
                    epilogue_tile(t, it)
                nc.sync.dma_start(
                    out=y_d.ap().rearrange("(t p) -> p t", p=P), in_=out_sb[:]
                )

            if repeat is None:
                body()
            else:
                with tc.For_i(0, repeat, 1) as _i:
                    body()

    nc.compile()
    return nc


def make_tab2(emb_table, W1):
    folded = np.asarray(emb_table, np.float32) @ np.asarray(W1, np.float32).T
    tab2 = np.zeros((NSUB * SUBROWS, ESZ), np.float32)
    for s in range(NSUB):
        tab2[s * SUBROWS: s * SUBROWS + SUBSZ, :H1] = folded[
            s * SUBSZ:(s + 1) * SUBSZ
        ]
    return tab2


_LAST_PERMS = None


def prep_in_maps(batch, lens, emb_table, W1, b1, W2, b2, W3, b3):
    """Returns (in_maps, plan) where plan = (calls, tot).  Also stashes the
    per-core row permutations in _LAST_PERMS (kernel() undoes them)."""
    global _LAST_PERMS
    batch = np.ascontiguousarray(np.asarray(batch, dtype=np.int32))
    lens_f = np.asarray(lens).astype(np.float32)
    recip = (np.float32(1.0) / lens_f).astype(np.float32)
    calls, tot, gidx_cores, perms = compute_plan(batch)
    _LAST_PERMS = perms
    common = {
        "tab2": make_tab2(emb_table, W1),
        "b1": np.asarray(b1, np.float32),
        "w2t": np.ascontiguousarray(np.asarray(W2, np.float32).T),
        "b2": np.asarray(b2, np.float32),
        "w3t": np.ascontiguousarray(np.asarray(W3, np.float32).T),
        "b3": np.asarray(b3, np.float32),
    }
    in_maps = []
    for c in range(NCORES):
        sl = slice(c * BC, (c + 1) * BC)
        in_maps.append(
            {"gidx": gidx_cores[c], "recip": recip[sl][perms[c]], **common}
        )
    return in_maps, (calls, tot)


_NC_CACHE = {}


def kernel(batch, lens, emb_table, W1, b1, W2, b2, W3, b3):
    from concourse.bass_utils import run_bass_kernel_spmd

    in_maps, (calls, tot) = prep_in_maps(
        batch, lens, emb_table, W1, b1, W2, b2, W3, b3
    )
    perms = _LAST_PERMS
    key = (calls, tot)
    if key not in _NC_CACHE:
        _NC_CACHE[key] = build_nc(calls, tot)
    nc = _NC_CACHE[key]
    last_err = None
    for _attempt in range(3):
        try:
            res = run_bass_kernel_spmd(nc, in_maps, core_ids=list(range(NCORES)))
            break
        except Exception as e:  # transient axon desync/device-state errors
            last_err = e
            import time as _time

            _time.sleep(5.0)
    else:
        raise last_err
    out = np.empty(B, np.float32)
    for c in range(NCORES):
        out[c * BC + perms[c]] = res.results[c]["y"]
    return out


# revision 21
# speedup vs baseline: 1.1631x; 1.1631x over previous
"""Trainium2 Bass kernel for nn_AvgPoolingModel (embedding avg-pool + tiny MLP).

Model:  emb = table[batch]           # [B, L, 300] gather
        pooled = emb.sum(1) / lens   # [B, 300]
        h1 = relu(pooled @ W1.T + b1)
        h2 = relu(h1 @ W2.T + b2)
        y  = (h2 @ W3.T + b3)[:, 0]  # [B]

Sharding: data-parallel over B across 8 cores (512 rows/core); folded
embedding table + MLP weights replicated per core.

Key transform: W1 is folded into the table HOST-side
(table2 = emb_table @ W1.T, [100000, 150] f32), which is exact
(pooled @ W1.T == (1/len)*sum table2[idx]) and cuts gathered bytes from
1200 to 600 per index.  Rows are padded to 192 f32 (768 B) for the 256 B
stride/payload rule of `dma_gather`.

Gather strategy (`dma_gather`, int16 indices, batched descriptors):
  - vocab split into 4 sub-ranges of 25000 rows so local indices fit
    int16; each sub-table gets 32 zero rows (pad reads spread across
    them to avoid single-HBM-channel congestion).
  - gathered list position j lands in partition j%128, slot j//128; the
    host orders each call's list so batch row p's indices stay in
    partition p.
  - HW limit (probed): <=1024 indices per call; each (row-tile,
    sub-range) splits into G balanced occurrence groups of width W<=8.
  - calls round-robin over 4 SWDGE queues (a single queue serializes
    descriptor generation behind DMA completion: 12.7us vs 1.9us/call).
  - batch rows are clustered into row-tiles by sub-range count profile
    (greedy), shrinking the rectangle width (max per-partition count)
    and with it the zero-row padding; the row permutation is undone
    host-side on the [B] output.

Per core: 4 row-tiles of gathers land packed into [128, 64*192] f32 ring
buffers (2 bufs); one DVE strided reduce per buffer (cols [:150])
collapses it into the row-tile accumulator.  Epilogue per tile: scale by
1/len, + b1 (rank-1 ones matmul in PSUM with an identity-matmul copy of
the scaled acc), relu, then the tiny W2/W3 matmuls with on-chip
transposes.
"""

import numpy as np

import concourse.bass as bass
import concourse.mybir as mybir
from concourse.tile import TileContext

VOCAB, EMB = 100000, 300
B, L = 4096, 200
H1, H2 = 150, 150
NCORES = 8
BC = B // NCORES  # rows per core
P = 128
NT = BC // P      # row-tiles per core
NSUB = 4
SUBSZ = VOCAB // NSUB
NZROW = 256           # spread pad reads over this many zero rows
SUBROWS = SUBSZ + NZROW
ZROW = SUBSZ
ESZ = 192             # table row stride (f32 elements), 768 B
PAYL = 192            # gathered payload per index (f32): full 768 B rows
                      # (HBM random access is transaction-bound: smaller
                      # payloads measured SLOWER, 608 B: 474us vs 768 B: 406us)
MAXW = 8              # max slots per partition per dma_gather call
SLOTBUF = 64          # ring buffer capacity in slots
NQ = 4                # SWDGE queues

F32 = mybir.dt.float32
I32 = mybir.dt.int32
I16 = mybir.dt.int16


def _dma_gather_raw(gp, out_ap, in_ap, idxs_ap, num_idxs, elem_size,
                    elem_step, queue_num=0):
    """dma_gather with payload not a multiple of 256 B (bass asserts that,
    but it is a transpose-only restriction: the non-transpose Q7
    descriptor generator sizes packets in raw bytes; only the stride --
    the idx multiplier -- is 256 B-granular in hardware).  Mirrors
    bass.dma_gather's non-transpose path."""
    from concourse import ap_utils
    from concourse.bass import MemorySpace, exact_div, round_up_to_multiple

    gp._assert_queue_num(queue_num)
    assert idxs_ap.dtype == mybir.dt.int16
    assert in_ap.space == MemorySpace.DRAM
    assert in_ap.dtype == out_ap.dtype
    assert idxs_ap.space == MemorySpace.SBUF
    assert out_ap.space == MemorySpace.SBUF
    assert ap_utils.ap_is_contiguous(out_ap.ap[1:])
    assert ap_utils.ap_is_contiguous(idxs_ap.ap[1:])
    assert num_idxs % 128 == 0
    assert out_ap.ap[0][1] * out_ap.ap[1][1] == round_up_to_multiple(num_idxs, 128)
    assert out_ap.ap[-1][1] == elem_size
    assert in_ap.ap[0][0] == elem_step
    dsz = mybir.dt.size(in_ap.dtype)
    stride_bytes_256 = exact_div(elem_step * dsz, 256)
    assert 0 < stride_bytes_256 < 256
    _in_ap = gp.lower_ap_dma(in_ap, for_custom_bir_dma=True)
    _idxs_ap = gp.lower_ap(idxs_ap)
    _out_ap = gp.lower_ap(out_ap)
    return gp.add_instruction(
        mybir.InstDMAGatherAnt(
            name=gp.bass.get_next_instruction_name(),
            ins=[*_in_ap, _idxs_ap, gp.lower_val_access(gp.to_reg(num_idxs))],
            outs=[_out_ap],
            transpose=False,
            num_idxs=num_idxs,
            elem_size=elem_size,
            stride_bytes_256=stride_bytes_256,
            gen_mode=0,
            single_packet=True,
            queue_num=queue_num,
            sbuf_tokens_per_rank=0,
            sbuf_free_dim_per_rank=0,
            sbuf_free_dim_pad_per_rank=0,
            sbuf_byte_offset=0,
        )
    )


def _cluster_rows(cnt):
    """Greedy: assign rows (count vectors [BC, NSUB]) to NT tiles of P rows,
    minimizing sum over tiles of per-sub-range max count.  Processing
    peaky rows first makes tile specializations line up across cores,
    which matters because call widths take a max over cores.  Returns
    perm (row order, tile-major)."""
    order = np.argsort(-cnt.max(axis=1), kind="stable")
    tiles = [[] for _ in range(NT)]
    tmax = np.zeros((NT, NSUB), np.int32)
    for r in order:
        best, bcost = None, None
        for t in range(NT):
            if len(tiles[t]) >= P:
                continue
            cost = int(np.maximum(tmax[t], cnt[r]).sum() - tmax[t].sum())
            if bcost is None or cost < bcost:
                best, bcost = t, cost
        tiles[best].append(r)
        tmax[best] = np.maximum(tmax[best], cnt[r])

    return np.concatenate([np.asarray(t, np.int64) for t in tiles])


def compute_plan(batch):
    """Host-side gather plan from the full [B, L] int index matrix.

    Returns (calls, tot, gidx_cores, perms):
      calls: tuple of (t, s, W, idx_off) in issue order; 128*W idx/call.
      tot:   per-partition int16 elements of the concatenated index tile.
      gidx_cores: list of NCORES [128, tot] int16 index tiles.
      perms: list of NCORES [BC] row permutations (device row i holds
             original row perm[i] of that core's slice).
    """
    batch = np.asarray(batch)
    sub = (batch // SUBSZ).astype(np.int8)
    loc = (batch - (batch // SUBSZ) * SUBSZ).astype(np.int16)

    perms = []
    groups = []   # groups[c][t][s][p] = array of local indices
    maxc = np.zeros((NT, NSUB), np.int32)
    for c in range(NCORES):
        r0 = c * BC
        cnt = np.stack(
            [(sub[r0:r0 + BC] == s).sum(axis=1) for s in range(NSUB)], axis=1
        ).astype(np.int32)
        perm = _cluster_rows(cnt)
        perms.append(perm)
        gc = [[[None] * P for _ in range(NSUB)] for _ in range(NT)]
        for t in range(NT):
            for p in range(P):
                rr = r0 + perm[t * P + p]
                rs, rl = sub[rr], loc[rr]
                for s in range(NSUB):
                    g = np.sort(rl[rs == s])
                    gc[t][s][p] = g
                    if len(g) > maxc[t, s]:
                        maxc[t, s] = len(g)
        groups.append(gc)

    calls = []
    off = 0
    for t in range(NT):
        for s in range(NSUB):
            mc = int(maxc[t, s])
            G = -(-mc // MAXW)  # ceil
            for g in range(G):
                W = min(MAXW, mc - MAXW * g)  # exact: sum of widths == maxc
                calls.append((t, s, g, G, W, off))
                off += 8 * W  # (128*W)/16 int16 elems per partition
    tot = off

    gidx_cores = []
    for c in range(NCORES):
        gidx = np.empty((P, tot), np.int16)
        padctr = 0
        for (t, s, g, G, W, ioff) in calls:
            arr = np.empty((P, W), np.int16)
            for p in range(P):
                gp = groups[c][t][s][p][MAXW * g: MAXW * g + W]
                n = len(gp)
                arr[p, :n] = gp
                npad = W - n
                if npad:
                    arr[p, n:] = ZROW + (padctr + np.arange(npad)) % NZROW
                    padctr += npad
            flat = arr.T.reshape(-1)               # flat[j], j = m*128 + p
            tile16 = flat.reshape(8 * W, 16).T     # [16, 8W]
            gidx[:, ioff: ioff + 8 * W] = np.tile(tile16, (8, 1))
        gidx_cores.append(gidx)

    call_keys = tuple((t, s, W, ioff) for (t, s, g, G, W, ioff) in calls)
    return call_keys, tot, gidx_cores, perms


def build_nc(calls, tot, repeat=None, gather_only=False):
    """Build the per-core Bass kernel for a gather plan.

    repeat=None: the real kernel.  repeat=R: the gather+reduce+MLP body is
    wrapped in a hardware For_i loop executing R times (identical work per
    iteration) — used only for wall-clock timing, where slope over R
    isolates HW exec time from the ~100ms axon dispatch overhead.
    """
    from concourse import bacc
    from concourse import library_config
    from concourse.masks import make_identity

    nc = bacc.Bacc("TRN2", target_bir_lowering=False, debug=False,
                   num_swdge_queues=NQ)

    gidx_d = nc.dram_tensor("gidx", [P, tot], I16, kind="ExternalInput")
    recip_d = nc.dram_tensor("recip", [BC], F32, kind="ExternalInput")
    tab_d = nc.dram_tensor("tab2", [NSUB * SUBROWS, ESZ], F32, kind="ExternalInput")
    b1_d = nc.dram_tensor("b1", [H1], F32, kind="ExternalInput")
    w2t_d = nc.dram_tensor("w2t", [H1, H2], F32, kind="ExternalInput")
    b2_d = nc.dram_tensor("b2", [H2], F32, kind="ExternalInput")
    w3t_d = nc.dram_tensor("w3t", [H2, 1], F32, kind="ExternalInput")
    b3_d = nc.dram_tensor("b3", [1], F32, kind="ExternalInput")
    y_d = nc.dram_tensor("y", [BC], F32, kind="ExternalOutput")

    # pack calls into ring buffers (per row-tile, <= SLOTBUF slots each)
    bufplans = {t: [] for t in range(NT)}  # t -> list of list[(s, W, ioff)]
    for (t, s, w, ioff) in calls:
        plans = bufplans[t]
        if not plans or sum(x[1] for x in plans[-1]) + w > SLOTBUF:
            plans.append([])
        plans[-1].append((s, w, ioff))

    with TileContext(nc) as tc:
        with (
            tc.tile_pool(name="const", bufs=1) as cpool,
            tc.tile_pool(name="gat", bufs=1) as gpool,
            tc.tile_pool(name="ring", bufs=3) as rpool,
            tc.tile_pool(name="work", bufs=2) as wpool,
            tc.tile_pool(name="psum", bufs=1, space="PSUM") as ppool,
            tc.tile_pool(name="psum2", bufs=2, space="PSUM") as ppool2,
        ):
            # ---- index tile (loaded first: it gates the gather stream) --
            gidx_sb = gpool.tile([P, tot], I16, tag="gidx", name="gidx_sb")
            tile_off = [tot]
            for (t, s, w, ioff) in calls:
                if t == 1:
                    tile_off[0] = ioff
                    break
            nc.sync.dma_start(out=gidx_sb[:, : tile_off[0]],
                              in_=gidx_d[:, : tile_off[0]])
            nc.sync.dma_start(out=gidx_sb[:, tile_off[0]:],
                              in_=gidx_d[:, tile_off[0]:])

            accs = []
            for t in range(NT):
                accs.append(gpool.tile([P, H1], F32, tag=f"acc{t}", name=f"acc{t}"))

            # ---- one-time constants -------------------------------------
            identity = cpool.tile([P, P], F32)
            make_identity(nc, identity[:])
            ones_row = cpool.tile([1, P], F32)
            nc.vector.memset(ones_row[:], 1.0)
            # gpsimd work above uses the standard library; everything after
            # this point on gpsimd is dma_gather (mlp library).
            nc.gpsimd.load_library(library_config.mlp)

            w2t_sb = cpool.tile([75, 2 * H2], F32)    # 2 K-chunks of W2.T
            for c in range(2):
                nc.sync.dma_start(
                    out=w2t_sb[:, c * H2:(c + 1) * H2],
                    in_=w2t_d[c * 75:(c + 1) * 75, :],
                )
            w3t_sb = cpool.tile([75, 2], F32)         # 2 K-chunks of W3.T
            for c in range(2):
                nc.sync.dma_start(
                    out=w3t_sb[:, c:c + 1], in_=w3t_d[c * 75:(c + 1) * 75, :]
                )
            b1_sb = cpool.tile([1, H1], F32)
            nc.sync.dma_start(out=b1_sb[:], in_=b1_d[None, :])
            b2_sb = cpool.tile([1, H2], F32)
            nc.sync.dma_start(out=b2_sb[:], in_=b2_d[None, :])
            b3_sb = cpool.tile([1, 1], F32)
            nc.sync.dma_start(out=b3_sb[:], in_=b3_d[None, :])

            recip_sb = cpool.tile([P, NT], F32)
            nc.sync.dma_start(
                out=recip_sb[:], in_=recip_d.ap().rearrange("(t p) -> p t", p=P)
            )
            out_sb = cpool.tile([P, NT], F32)

            qctr = [0]

            # ---- per-row-tile: batched gathers + packed DVE reduces -----
            def gather_tile(t, it=""):
                first = True
                for bi, plan in enumerate(bufplans[t]):
                    nslots = sum(w for (_s, w, _o) in plan)
                    ring = rpool.tile([P, SLOTBUF * PAYL], F32, tag="ring",
                                      name=f"ring{t}_{bi}{it}")
                    col = 0
                    for (s, w, ioff) in plan:
                        _dma_gather_raw(
                            nc.gpsimd,
                            ring[:, col * PAYL:(col + w) * PAYL].rearrange(
                                "p (w e) -> p w e", e=PAYL
                            ),
                            tab_d[s * SUBROWS:(s + 1) * SUBROWS, :],
                            gidx_sb[:, ioff: ioff + 8 * w],
                            P * w,
                            PAYL,
                            ESZ,
                            queue_num=qctr[0] % NQ,
                        )
                        qctr[0] += 1
                        col += w
                    rview = ring[:, : nslots * PAYL].rearrange(
                        "p (w e) -> p e w", e=PAYL
                    )[:, :H1, :]
                    if first:
                        nc.vector.reduce_sum(
                            accs[t][:], rview, axis=mybir.AxisListType.X
                        )
                        first = False
                    else:
                        part = wpool.tile([P, H1], F32, tag="part",
                                          name=f"part{t}_{bi}{it}")
                        nc.vector.reduce_sum(
                            part[:], rview, axis=mybir.AxisListType.X
                        )
                        nc.vector.tensor_add(
                            out=accs[t][:], in0=accs[t][:], in1=part[:]
                        )

            # ---- per-row-tile epilogue: scale, +b1, relu, W2, W3 --------
            def epilogue_tile(t, it=""):
                acc = accs[t]
                scaled = wpool.tile([P, H1], F32, tag="scaled",
                                    name=f"scaled{t}{it}")
                nc.vector.tensor_scalar_mul(
                    scaled[:], acc[:], recip_sb[:, t:t + 1]
                )

                # h1 = relu(scaled + b1): PSUM gets identity-matmul copy of
                # scaled, then the rank-1 ones x b1 matmul adds the bias.
                h1_ps = ppool.tile([P, H1], F32, tag="h1", name=f"h1ps{t}{it}")
                nc.tensor.matmul(
                    out=h1_ps[:], lhsT=identity[:], rhs=scaled[:],
                    start=True, stop=False,
                )
                nc.tensor.matmul(
                    out=h1_ps[:], lhsT=ones_row[:], rhs=b1_sb[:],
                    start=False, stop=True,
                )
                h1_sb = wpool.tile([P, H1], F32, tag="h1sb", name=f"h1sb{t}{it}")
                nc.scalar.activation(
                    h1_sb[:], h1_ps[:], mybir.ActivationFunctionType.Relu
                )

                h1t = wpool.tile([75, 2 * P], F32, tag="h1t", name=f"h1t{t}{it}")
                for c in range(2):
                    t1_ps = ppool2.tile([75, P], F32, tag="tps",
                                        name=f"t1{t}_{c}{it}")
                    nc.tensor.transpose(
                        out=t1_ps[:], in_=h1_sb[:, c * 75:(c + 1) * 75],
                        identity=identity[:],
                    )
                    nc.scalar.copy(h1t[:, c * P:(c + 1) * P], t1_ps[:])

                h2_ps = ppool.tile([P, H2], F32, tag="h2", name=f"h2ps{t}{it}")
                for c in range(2):
                    nc.tensor.matmul(
                        out=h2_ps[:],
                        lhsT=h1t[:, c * P:(c + 1) * P],
                        rhs=w2t_sb[:, c * H2:(c + 1) * H2],
                        start=(c == 0), stop=False,
                    )
                nc.tensor.matmul(
                    out=h2_ps[:], lhsT=ones_row[:], rhs=b2_sb[:],
                    start=False, stop=True,
                )
                h2_sb = wpool.tile([P, H2], F32, tag="h2sb", name=f"h2sb{t}{it}")
                nc.scalar.activation(
                    h2_sb[:], h2_ps[:], mybir.ActivationFunctionType.Relu
                )

                h2t = wpool.tile([75, 2 * P], F32, tag="h2t", name=f"h2t{t}{it}")
                for c in range(2):
                    t2_ps = ppool2.tile([75, P], F32, tag="tps",
                                        name=f"t2{t}_{c}{it}")
                    nc.tensor.transpose(
                        out=t2_ps[:], in_=h2_sb[:, c * 75:(c + 1) * 75],
                        identity=identity[:],
                    )
                    nc.scalar.copy(h2t[:, c * P:(c + 1) * P], t2_ps[:])

                y_ps = ppool.tile([P, 1], F32, tag="y", name=f"yps{t}{it}")
                for c in range(2):
                    nc.tensor.matmul(
                        out=y_ps[:],
                        lhsT=h2t[:, c * P:(c + 1) * P],
                        rhs=w3t_sb[:, c:c + 1],
                        start=(c == 0), stop=False,
                    )
                nc.tensor.matmul(
                    out=y_ps[:], lhsT=ones_row[:], rhs=b3_sb[:],
                    start=False, stop=True,
                )
                nc.scalar.copy(out_sb[:, t:t + 1], y_ps[:])

            def body(it=""):
                if gather_only:
                    # gathers + a token DVE consumer per buffer (copy of one
                    # column) so Tile keeps real DMA-completion deps without
                    # the full reduce cost.
                    for t in range(NT):
                        for bi, plan in enumerate(bufplans[t]):
                            ring = rpool.tile([P, SLOTBUF * PAYL], F32,
                                              tag="ring", name=f"ring{t}_{bi}{it}")
                            col = 0
                            for (s, w, ioff) in plan:
                                _dma_gather_raw(
                                    nc.gpsimd,
                                    ring[:, col * PAYL:(col + w) * PAYL].rearrange(
                                        "p (w e) -> p w e", e=PAYL
                                    ),
                                    tab_d[s * SUBROWS:(s + 1) * SUBROWS, :],
                                    gidx_sb[:, ioff: ioff + 8 * w],
                                    P * w, PAYL, ESZ,
                                    queue_num=qctr[0] % NQ,
                                )
                                qctr[0] += 1
                                col += w
                            nc.vector.tensor_copy(
                                out=out_sb[:, t:t + 1],
                                in_=ring[:, (col - 1) * PAYL:(col - 1) * PAYL + 1],
                            )
                    nc.sync.dma_start(
                        out=y_d.ap().rearrange("(t p) -> p t", p=P), in_=out_sb[:]
                    )
                    return
                for t in range(NT):
                    gather_tile(t, it)
                    epilogue_tile(t, it)
                nc.sync.dma_start(
                    out=y_d.ap().rearrange("(t p) -> p t", p=P), in_=out_sb[:]
                )

            if repeat is None:
                body()
            else:
                with tc.For_i(0, repeat, 1) as _i:
                    body()

    nc.compile()
    return nc


def make_tab2(emb_table, W1):
    folded = np.asarray(emb_table, np.float32) @ np.asarray(W1, np.float32).T
    tab2 = np.zeros((NSUB * SUBROWS, ESZ), np.float32)
    for s in range(NSUB):
        tab2[s * SUBROWS: s * SUBROWS + SUBSZ, :H1] = folded[
            s * SUBSZ:(s + 1) * SUBSZ
        ]
    return tab2


_LAST_PERMS = None


def prep_in_maps(batch, lens, emb_table, W1, b1, W2, b2, W3, b3):
    """Returns (in_maps, plan) where plan = (calls, tot).  Also stashes the
    per-core row permutations in _LAST_PERMS (kernel() undoes them)."""
    global _LAST_PERMS
    batch = np.ascontiguousarray(np.asarray(batch, dtype=np.int32))
    lens_f = np.asarray(lens).astype(np.float32)
    recip = (np.float32(1.0) / lens_f).astype(np.float32)
    calls, tot, gidx_cores, perms = compute_plan(batch)
    _LAST_PERMS = perms
    common = {
        "tab2": make_tab2(emb_table, W1),
        "b1": np.asarray(b1, np.float32),
        "w2t": np.ascontiguousarray(np.asarray(W2, np.float32).T),
        "b2": np.asarray(b2, np.float32),
        "w3t": np.ascontiguousarray(np.asarray(W3, np.float32).T),
        "b3": np.asarray(b3, np.float32),
    }
    in_maps = []
    for c in range(NCORES):
        sl = slice(c * BC, (c + 1) * BC)
        in_maps.append(
            {"gidx": gidx_cores[c], "recip": recip[sl][perms[c]], **common}
        )
    return in_maps, (calls, tot)


_NC_CACHE = {}


def kernel(batch, lens, emb_table, W1, b1, W2, b2, W3, b3):
    from concourse.bass_utils import run_bass_kernel_spmd

    in_maps, (calls, tot) = prep_in_maps(
        batch, lens, emb_table, W1, b1, W2, b2, W3, b3
    )
    perms = _LAST_PERMS
    key = (calls, tot)
    if key not in _NC_CACHE:
        _NC_CACHE[key] = build_nc(calls, tot)
    nc = _NC_CACHE[key]
    last_err = None
    for _attempt in range(3):
        try:
            res = run_bass_kernel_spmd(nc, in_maps, core_ids=list(range(NCORES)))
            break
        except Exception as e:  # transient axon desync/device-state errors
            last_err = e
            import time as _time

            _time.sleep(5.0)
    else:
        raise last_err
    out = np.empty(B, np.float32)
    for c in range(NCORES):
        out[c * BC + perms[c]] = res.results[c]["y"]
    return out
